# revision 29
# baseline (speedup 1.0000x reference)
"""Biased axial attention on 8 TRN2 NeuronCores (Bass/Tile SPMD kernel).

Sharding: leading (non-attended) L axis n across 8 cores (sequence parallel).
QK logits are partial-summed per core, bias chunk (i-sharded) enters via an
early AllGather + fused add on the logit drain, one AllReduce of the
[h,i,j] logits, softmax redundantly per core, AV/gating/output local in n.

Compute dtype: bf16 operands, fp32 PSUM accumulation. Logits/softmax in fp32.
"""

import math
import numpy as np
import ml_dtypes

import concourse.bass as bass
import concourse.bacc as bacc
import concourse.tile as tile
from concourse import mybir
from concourse.bass_utils import run_bass_kernel_spmd

BF16 = mybir.dt.bfloat16
F32 = mybir.dt.float32
NPBF16 = ml_dtypes.bfloat16

LAST_RESULT = None  # BassKernelResults of the most recent run (for test.py)

NCORES = 8
L = 384
DP = 128  # pair channels
DB = 128  # bias channels
H = 4
D = 32
HD = H * D  # 128
NLOC = L // NCORES  # 48 rows per core
TC = 3  # token chunks of 128
EPS = 1e-5
AF = mybir.ActivationFunctionType


def _bias_tile_subranges(ic):
    """For logits i-chunk ic (global i = ic*128 + p, p in 0..127), return the
    (partition_start, core_block, row_start, nrows) runs mapping partitions to
    the AllGather layout bgath[core_block, h, row, :] (48 rows per block)."""
    runs = []
    p = 0
    while p < 128:
        i = ic * 128 + p
        c2, il = divmod(i, NLOC)
        nrows = min(NLOC - il, 128 - p)
        runs.append((p, c2, il, nrows))
        p += nrows
    return runs


def build_program(wq, wk, wv, wg, wb, wout, qb, kb, vb, gbf, bb, bout):
    """Build the SPMD Bass program. Weight args are numpy f32 host copies used
    only to decide which (usually-zero) bias paths to emit."""
    has_qb = bool(np.any(qb != 0))
    has_kb = bool(np.any(kb != 0))
    has_vb = bool(np.any(vb != 0))
    has_bb = bool(np.any(bb != 0))
    has_bout = bool(np.any(bout != 0))

    nc = bacc.Bacc(
        "TRN2",
        target_bir_lowering=False,
        debug=False,
        enable_asserts=False,
        num_devices=NCORES,
    )

    # ------------------------------------------------------------------ I/O
    pair_s = nc.dram_tensor("pair_s", [NLOC, L, DP], BF16, kind="ExternalInput").ap()
    bias_s = nc.dram_tensor("bias_s", [NLOC, L, DB], BF16, kind="ExternalInput").ap()
    wq_d = nc.dram_tensor("wq", [DP, HD], BF16, kind="ExternalInput").ap()
    wk_d = nc.dram_tensor("wk", [DP, HD], BF16, kind="ExternalInput").ap()
    wv_d = nc.dram_tensor("wv", [DP, HD], BF16, kind="ExternalInput").ap()
    wg_d = nc.dram_tensor("wg", [DP, HD], BF16, kind="ExternalInput").ap()
    wb_d = nc.dram_tensor("wb", [DB, 32], BF16, kind="ExternalInput").ap()
    wout_d = nc.dram_tensor("wout", [HD, DP], BF16, kind="ExternalInput").ap()
    # small fp32 vectors, packed on host:
    # cvec[:, 0]=qb_pk, 1=kb_pk, 2=gbf, 3=bb_pk, 4=bout, 5=ones
    cvec_d = nc.dram_tensor("cvec", [128, 6], F32, kind="ExternalInput").ap()
    vbrow_d = nc.dram_tensor("vbrow", [1, TC * HD], F32, kind="ExternalInput").ap()
    boutrow_d = nc.dram_tensor("boutrow", [1, DP], BF16, kind="ExternalInput").ap()
    ident_d = nc.dram_tensor("ident", [128, 128], BF16, kind="ExternalInput").ap()
    out_d = nc.dram_tensor("out", [NLOC, L, DP], F32, kind="ExternalOutput").ap()

    with tile.TileContext(nc) as tc, tc.tile_pool(name="persist", bufs=1) as pp:
        # ------------------------------------------------------- persistent SBUF
        wq_sb = pp.tile([DP, HD], BF16, name="wq_sb")
        wk_sb = pp.tile([DP, HD], BF16, name="wk_sb")
        wv_sb = pp.tile([DP, HD], BF16, name="wv_sb")
        wg_sb = pp.tile([DP, HD], BF16, name="wg_sb")
        wb_sb = pp.tile([DB, 32], BF16, name="wb_sb")
        wout_sb = pp.tile([HD, DP], BF16, name="wout_sb")
        cvec_sb = pp.tile([128, 6], F32, name="cvec_sb")
        vb_sb = pp.tile([128, TC * HD], F32, name="vb_sb")
        bout_sb = pp.tile([1, DP], BF16, name="bout_sb")
        ones_sb = pp.tile([1, 128], BF16, name="ones_sb")
        eps_sb = pp.tile([128, 1], F32, name="eps_sb")
        ident_sb = pp.tile([128, 128], BF16, name="ident_sb")

        nc.gpsimd.dma_start(out=wq_sb[:], in_=wq_d[:])
        nc.gpsimd.dma_start(out=wk_sb[:], in_=wk_d[:])
        nc.gpsimd.dma_start(out=wv_sb[:], in_=wv_d[:])
        nc.gpsimd.dma_start(out=wg_sb[:], in_=wg_d[:])
        nc.gpsimd.dma_start(out=wb_sb[:], in_=wb_d[:])
        nc.gpsimd.dma_start(out=wout_sb[:], in_=wout_d[:])
        nc.gpsimd.dma_start(out=cvec_sb[:], in_=cvec_d[:])
        nc.vector.memset(eps_sb[:], EPS)
        nc.sync.dma_start(out=ident_sb[:], in_=ident_d[:])
        if has_vb:
            nc.gpsimd.dma_start(out=vb_sb[:], in_=vbrow_d.to_broadcast((128, TC * HD)))
        if has_bout:
            nc.gpsimd.dma_start(out=bout_sb[:], in_=boutrow_d[:])
            nc.vector.memset(ones_sb[:], 1.0)

        qpk_sb = pp.tile([128, 12, H, L], BF16, name="qpk_sb")   # [(ns,d), g, h, i]
        kpk_sb = pp.tile([128, 12, H, L], BF16, name="kpk_sb")
        v_sb = pp.tile([128, NLOC, TC, HD], BF16, name="v_sb")   # [j_in_chunk, n, jc, hd]
        g_sb = pp.tile([128, NLOC, L], BF16, name="g_sb")        # [(h,d), n, i] pre-sigmoid
        a_sb = pp.tile([128, H, TC, L], BF16, name="a_sb")       # [i_in_chunk, h, ic, j]
        at_sb = pp.tile([128, H, TC, L], BF16, name="at_sb")     # [j_in_chunk, h, jc, i]
        sums_sb = pp.tile([128, H, TC], F32, name="sums_sb")
        recip_sb = pp.tile([128, H, TC], F32, name="recip_sb")
        DEFER = 16  # rows whose v/g compute fills the AllReduce window
        pnk_sb = pp.tile([128, DEFER, L], BF16, name="pnk_sb")

        # ------------------------------------------------------------- DRAM
        with tc.tile_pool(name="dram", bufs=1, space="DRAM") as dram:
            bchunk = dram.tile([H, NLOC, L], F32)                 # this core's bias rows
            bgath = dram.tile([NCORES, H, NLOC, L], F32, addr_space="Shared")
            bounce_in = dram.tile([H, L, L], F32)
            bounce_out0 = dram.tile([2, L, L], F32, addr_space="Shared")
            bounce_out1 = dram.tile([2, L, L], F32, addr_space="Shared")

            # =======================================================
            # Phase B: bias path (i-sharded rows), feeds AllGather
            # =======================================================
            with (
                tc.tile_pool(name="bxt", bufs=6) as bxt_pool,
                tc.tile_pool(name="bxn", bufs=3) as bxn_pool,
                tc.tile_pool(name="bst", bufs=2) as bst_pool,
                tc.tile_pool(name="bnt", bufs=6) as bnt_pool,
                tc.tile_pool(name="bsb", bufs=2) as bsb_pool,
                tc.tile_pool(name="psB", bufs=2, space="PSUM") as psB,
                tc.tile_pool(name="psTB", bufs=2, space="PSUM") as psTB,
            ):
                for k4 in range(12):  # 4 bias rows per PSUM bank
                    st = bst_pool.tile([128, 4, TC, 6], F32, name="st_b")
                    mv = bst_pool.tile([128, 4, TC, 2], F32, name="mv_b")
                    rstd = bst_pool.tile([128, 4, TC], F32, name="rstd_b")
                    nmr = bst_pool.tile([128, 4, TC], F32, name="nmr_b")
                    xts = []
                    for ii in range(4):
                        i = 4 * k4 + ii
                        xt = bxt_pool.tile([128, TC, DB], BF16, name="xt_b")
                        xts.append(xt)
                        nc.sync.dma_start(
                            out=xt[:],
                            in_=bias_s[i].rearrange("(c p) d -> p c d", p=128),
                        )
                        for j in range(TC):
                            nc.vector.bn_stats(out=st[:, ii, j], in_=xt[:, j])
                            nc.vector.bn_aggr(out=mv[:, ii, j], in_=st[:, ii, j])
                    nc.scalar.activation(
                        out=rstd[:], in_=mv[:, :, :, 1], func=AF.Sqrt,
                        bias=eps_sb[:], scale=1.0,
                    )
                    nc.vector.reciprocal(out=rstd[:], in_=rstd[:])
                    nc.vector.scalar_tensor_tensor(
                        out=nmr[:], in0=mv[:, :, :, 0], scalar=-1.0,
                        in1=rstd[:], op0=mybir.AluOpType.mult,
                        op1=mybir.AluOpType.mult,
                    )
                    ps_b = psB.tile([128, L], F32, name="ps_b")
                    for ii in range(4):
                        i = 4 * k4 + ii
                        xn = bxn_pool.tile([128, TC, DB], BF16, name="xn_b")
                        bnT = bnt_pool.tile([128, L], BF16, name="bnT")
                        psT = psTB.tile([128, L], BF16, name="psT_b")
                        for j in range(TC):
                            nc.vector.tensor_scalar(
                                out=xn[:, j], in0=xts[ii][:, j],
                                scalar1=nmr[:, ii, j : j + 1], scalar2=None,
                                op0=mybir.AluOpType.add,
                            )
                            nc.vector.tensor_scalar(
                                out=xn[:, j], in0=xn[:, j],
                                scalar1=rstd[:, ii, j : j + 1], scalar2=None,
                                op0=mybir.AluOpType.mult,
                            )
                            nc.tensor.transpose(
                                out=psT[:, j * 128 : (j + 1) * 128],
                                in_=xn[:, j], identity=ident_sb[:],
                            )
                        nc.vector.tensor_copy(out=bnT[:], in_=psT[:])
                        # braw[h, j] for this row at partitions 32*ii..32*ii+4
                        nc.tensor.matmul(
                            out=ps_b[32 * ii : 32 * ii + 32, :],
                            lhsT=wb_sb[:], rhs=bnT[:],
                            start=True, stop=True,
                            tile_position=(0, 32 * ii),
                        )
                    b_sb = bsb_pool.tile([128, L], F32, name="b_sb")
                    if has_bb:
                        nc.any.tensor_scalar(
                            out=b_sb[:], in0=ps_b[:], scalar1=cvec_sb[:, 3:4],
                            scalar2=None, op0=mybir.AluOpType.add,
                        )
                    else:
                        nc.any.tensor_copy(out=b_sb[:], in_=ps_b[:])
                    # rows {32*ii + h} -> bchunk[h, 4*k4+ii, :]
                    for ii in range(4):
                        nc.gpsimd.dma_start(
                            out=bchunk[:, 4 * k4 + ii, :],
                            in_=b_sb[32 * ii : 32 * ii + H, :],
                        )

            nc.gpsimd.collective_compute(
                "AllGather",
                mybir.AluOpType.bypass,
                replica_groups=[list(range(NCORES))],
                ins=[bchunk[:].opt()],
                outs=[bgath[:].opt()],
            )

            # =======================================================
            # Phase A: main rows -> pnT -> packed q/k slabs, v, g_pre
            # =======================================================
            vg_pool_ctx = tc.tile_pool(name="psVG", bufs=2, space="PSUM")
            psVG = vg_pool_ctx.__enter__()

            def emit_vg(n, pnT_ap):
                ps_v = psVG.tile([128, TC * HD], F32, name="ps_vg", tag="vg")
                for j in range(TC):
                    nc.tensor.matmul(
                        out=ps_v[:, j * HD : (j + 1) * HD],
                        lhsT=pnT_ap[:, j * 128 : (j + 1) * 128],
                        rhs=wv_sb[:], start=True, stop=True,
                    )
                if has_vb:
                    nc.vector.tensor_tensor(
                        out=v_sb[:, n], in0=ps_v[:], in1=vb_sb[:],
                        op=mybir.AluOpType.add,
                    )
                else:
                    nc.scalar.copy(out=v_sb[:, n], in_=ps_v[:])
                ps_g = psVG.tile([128, L], F32, name="ps_vg2", tag="vg")
                nc.tensor.matmul(
                    out=ps_g[:], lhsT=wg_sb[:], rhs=pnT_ap[:],
                    start=True, stop=True,
                )
                nc.scalar.copy(out=g_sb[:, n], in_=ps_g[:])

            with (
                tc.tile_pool(name="axt", bufs=6) as axt_pool,
                tc.tile_pool(name="axn", bufs=3) as axn_pool,
                tc.tile_pool(name="ast", bufs=2) as ast_pool,
                tc.tile_pool(name="pnt", bufs=6) as pnt_pool,
                tc.tile_pool(name="psTA", bufs=2, space="PSUM") as psTA,
                tc.tile_pool(name="psQ", bufs=2, space="PSUM") as psQ,
                tc.tile_pool(name="psK", bufs=2, space="PSUM") as psK,
            ):
                for g in range(12):
                    st = ast_pool.tile([128, 4, TC, 6], F32, name="st_a")
                    mv = ast_pool.tile([128, 4, TC, 2], F32, name="mv_a")
                    rstd = ast_pool.tile([128, 4, TC], F32, name="rstd_a")
                    nmr = ast_pool.tile([128, 4, TC], F32, name="nmr_a")
                    xts = []
                    for ns in range(4):
                        n = 4 * g + ns
                        xt = axt_pool.tile([128, TC, DP], BF16, name="xt_a")
                        xts.append(xt)
                        nc.sync.dma_start(
                            out=xt[:],
                            in_=pair_s[n].rearrange("(c p) d -> p c d", p=128),
                        )
                        for j in range(TC):
                            nc.vector.bn_stats(out=st[:, ns, j], in_=xt[:, j])
                            nc.vector.bn_aggr(out=mv[:, ns, j], in_=st[:, ns, j])
                    nc.scalar.activation(
                        out=rstd[:], in_=mv[:, :, :, 1], func=AF.Sqrt,
                        bias=eps_sb[:], scale=1.0,
                    )
                    nc.vector.reciprocal(out=rstd[:], in_=rstd[:])
                    nc.vector.scalar_tensor_tensor(
                        out=nmr[:], in0=mv[:, :, :, 0], scalar=-1.0,
                        in1=rstd[:], op0=mybir.AluOpType.mult,
                        op1=mybir.AluOpType.mult,
                    )
                    pnts = []
                    for ns in range(4):
                        n = 4 * g + ns
                        xn = axn_pool.tile([128, TC, DP], BF16, name="xn_a")
                        pnT = pnt_pool.tile([128, L], BF16, name="pnT")
                        pnts.append(pnT)
                        psT = psTA.tile([128, L], BF16, name="psT_a")
                        for j in range(TC):
                            nc.vector.tensor_scalar(
                                out=xn[:, j], in0=xts[ns][:, j],
                                scalar1=nmr[:, ns, j : j + 1], scalar2=None,
                                op0=mybir.AluOpType.add,
                            )
                            nc.vector.tensor_scalar(
                                out=xn[:, j], in0=xn[:, j],
                                scalar1=rstd[:, ns, j : j + 1], scalar2=None,
                                op0=mybir.AluOpType.mult,
                            )
                            nc.tensor.transpose(
                                out=psT[:, j * 128 : (j + 1) * 128],
                                in_=xn[:, j], identity=ident_sb[:],
                            )
                        nc.vector.tensor_copy(out=pnT[:], in_=psT[:])
                        if n >= NLOC - 16:
                            # defer v/g: stash pnT, compute during the AllReduce
                            nc.vector.tensor_copy(
                                out=pnk_sb[:, n - (NLOC - 16)], in_=pnT[:]
                            )
                        else:
                            emit_vg(n, pnT)
                    # packed q/k slabs for this group
                    for h in range(H):
                        ps_q = psQ.tile([128, L], F32, name="ps_q")
                        ps_k = psK.tile([128, L], F32, name="ps_k")
                        for ns in range(4):
                            nc.tensor.matmul(
                                out=ps_q[32 * ns : 32 * ns + 32, :],
                                lhsT=wq_sb[:, h * D : (h + 1) * D],
                                rhs=pnts[ns][:], start=True, stop=True,
                                tile_position=(0, 32 * ns),
                            )
                        for ns in range(4):
                            nc.tensor.matmul(
                                out=ps_k[32 * ns : 32 * ns + 32, :],
                                lhsT=wk_sb[:, h * D : (h + 1) * D],
                                rhs=pnts[ns][:], start=True, stop=True,
                                tile_position=(0, 32 * ns),
                            )
                        if has_qb:
                            nc.scalar.activation(
                                out=qpk_sb[:, g, h], in_=ps_q[:], func=AF.Copy,
                                bias=0.0, scale=1.0,
                            )
                            nc.vector.tensor_scalar(
                                out=qpk_sb[:, g, h], in0=qpk_sb[:, g, h],
                                scalar1=cvec_sb[:, 0:1], scalar2=None,
                                op0=mybir.AluOpType.add,
                            )
                        else:
                            nc.scalar.copy(out=qpk_sb[:, g, h], in_=ps_q[:])
                        if has_kb:
                            nc.vector.tensor_scalar(
                                out=kpk_sb[:, g, h], in0=ps_k[:],
                                scalar1=cvec_sb[:, 1:2], scalar2=None,
                                op0=mybir.AluOpType.add,
                            )
                        else:
                            nc.vector.tensor_copy(out=kpk_sb[:, g, h], in_=ps_k[:])

            # =======================================================
            # Phase C1: QK logits + fused bias add, write bounce
            # =======================================================
            with (
                tc.tile_pool(name="psL", bufs=2, space="PSUM") as psL,
                tc.tile_pool(name="btile", bufs=2) as btile_pool,
                tc.tile_pool(name="ldr", bufs=2) as ldr_pool,
                tc.tile_pool(name="attin", bufs=3) as attin_pool,
                tc.tile_pool(name="psTC", bufs=2, space="PSUM") as psTC,
            ):
                bounce_outs = [bounce_out0, bounce_out1]
                for half in range(2):
                    for h in (2 * half, 2 * half + 1):
                        for ic in range(TC):
                            bias_t = btile_pool.tile([128, L], F32, name="bias_t")
                            for (p0, c2, il, nr) in _bias_tile_subranges(ic):
                                nc.gpsimd.dma_start(
                                    out=bias_t[p0 : p0 + nr, :],
                                    in_=bgath[c2, h, il : il + nr, :],
                                )
                            ps_l = psL.tile([128, L], F32, name="ps_l")
                            for g in range(12):
                                nc.tensor.matmul(
                                    out=ps_l[:],
                                    lhsT=qpk_sb[:, g, h, ic * 128 : (ic + 1) * 128],
                                    rhs=kpk_sb[:, g, h],
                                    start=(g == 0), stop=(g == 11),
                                )
                            ldrain = ldr_pool.tile([128, L], F32, name="ldrain")
                            nc.vector.tensor_tensor(
                                out=ldrain[:], in0=ps_l[:], in1=bias_t[:],
                                op=mybir.AluOpType.add,
                            )
                            nc.sync.dma_start(
                                out=bounce_in[h, ic * 128 : (ic + 1) * 128, :],
                                in_=ldrain[:],
                            )
                    nc.gpsimd.collective_compute(
                        "AllReduce",
                        mybir.AluOpType.add,
                        replica_groups=[list(range(NCORES))],
                        ins=[bounce_in[2 * half : 2 * half + 2].opt()],
                        outs=[bounce_outs[half][:].opt()],
                    )

            # deferred v/g for the last rows (overlaps the AllReduce)
            for dn in range(16):
                emit_vg(NLOC - 16 + dn, pnk_sb[:, dn])
            vg_pool_ctx.__exit__(None, None, None)

            # batched sigmoid on g (overlaps the first AllReduce)
            for q4 in range(4):
                nc.scalar.activation(
                    out=g_sb[:, q4 * 12 : (q4 + 1) * 12],
                    in_=g_sb[:, q4 * 12 : (q4 + 1) * 12],
                    func=AF.Sigmoid, bias=cvec_sb[:, 2:3], scale=1.0,
                )

            # =======================================================
            # Phase C2: softmax (redundant on each core), per AR half
            # =======================================================
            with (
                tc.tile_pool(name="attin2", bufs=3) as attin_pool,
                tc.tile_pool(name="psTC2", bufs=2, space="PSUM") as psTC,
            ):
                bounce_outs = [bounce_out0, bounce_out1]
                for h in range(H):
                    for ic in range(TC):
                        att = attin_pool.tile([128, L], F32, name="att")
                        nc.sync.dma_start(
                            out=att[:],
                            in_=bounce_outs[h // 2][h % 2,
                                                    ic * 128 : (ic + 1) * 128, :],
                        )
                        nc.scalar.activation(
                            out=a_sb[:, h, ic], in_=att[:], func=AF.Exp,
                            bias=0.0, scale=1.0,
                        )
                        nc.vector.tensor_reduce(
                            out=sums_sb[:, h, ic : ic + 1], in_=a_sb[:, h, ic],
                            axis=mybir.AxisListType.X, op=mybir.AluOpType.add,
                        )
                    nc.vector.reciprocal(
                        out=recip_sb[:, h], in_=sums_sb[:, h]
                    )
                    for ic in range(TC):
                        nc.vector.tensor_scalar(
                            out=a_sb[:, h, ic], in0=a_sb[:, h, ic],
                            scalar1=recip_sb[:, h, ic : ic + 1], scalar2=None,
                            op0=mybir.AluOpType.mult,
                        )
                    for jc in range(TC):
                        psT = psTC.tile([128, L], BF16, name="psT_c")
                        for ic in range(TC):
                            nc.tensor.transpose(
                                out=psT[:, ic * 128 : (ic + 1) * 128],
                                in_=a_sb[:, h, ic, jc * 128 : (jc + 1) * 128],
                                identity=ident_sb[:],
                            )
                        nc.vector.tensor_copy(out=at_sb[:, h, jc], in_=psT[:])

            # =======================================================
            # Phase D: AV, gating, output projection
            # =======================================================
            with (
                tc.tile_pool(name="psO", bufs=4, space="PSUM") as psO,
                tc.tile_pool(name="psF", bufs=3, space="PSUM") as psF,
                tc.tile_pool(name="gO", bufs=5) as gO_pool,
                tc.tile_pool(name="osb", bufs=2) as osb_pool,
            ):
                for nb in range(12):
                    gOs = []
                    for nn in range(4):
                        n = 4 * nb + nn
                        ps_o = psO.tile([128, L], F32, name="ps_o")
                        for jc in range(TC):
                            for h in range(H):
                                nc.tensor.matmul(
                                    out=ps_o[32 * h : 32 * h + 32, :],
                                    lhsT=v_sb[:, n, jc, h * D : (h + 1) * D],
                                    rhs=at_sb[:, h, jc],
                                    start=(jc == 0),
                                    stop=(jc == TC - 1),
                                    tile_position=(0, 32 * h),
                                    skip_group_check=True,
                                )
                        gO = gO_pool.tile([128, L], BF16, name="gO")
                        gOs.append(gO)
                        nc.vector.tensor_tensor(
                            out=gO[:], in0=ps_o[:], in1=g_sb[:, n],
                            op=mybir.AluOpType.mult,
                        )
                    for jt in range(TC):
                        ps_f = psF.tile([128, 4 * DP], F32, name="ps_f")
                        for nn in range(4):
                            nc.tensor.matmul(
                                out=ps_f[:, nn * DP : (nn + 1) * DP],
                                lhsT=gOs[nn][:, jt * 128 : (jt + 1) * 128],
                                rhs=wout_sb[:], start=True, stop=not has_bout,
                            )
                            if has_bout:
                                nc.tensor.matmul(
                                    out=ps_f[:, nn * DP : (nn + 1) * DP],
                                    lhsT=ones_sb[:],
                                    rhs=bout_sb[:],
                                    start=False, stop=True,
                                )
                        out_sb = osb_pool.tile([128, 4 * DP], F32, name="out_sb")
                        nc.scalar.copy(out=out_sb[:], in_=ps_f[:])
                        nc.sync.dma_start(
                            out=out_d[4 * nb : 4 * nb + 4,
                                      jt * 128 : (jt + 1) * 128, :]
                            .rearrange("n t d -> t n d"),
                            in_=out_sb.rearrange("t (n d) -> t n d", n=4),
                        )

    return nc


def prepare(pair, bias, gamma_p, beta_p, gamma_b, beta_b,
            Wq, Wk, Wv, Wb, Wg, bg, Wout, bout):
    """Fold weights, build the program, shard inputs. Returns (nc, in_maps)."""
    pair = np.asarray(pair, np.float32)
    bias = np.asarray(bias, np.float32)
    gamma_p = np.asarray(gamma_p, np.float32)
    beta_p = np.asarray(beta_p, np.float32)
    gamma_b = np.asarray(gamma_b, np.float32)
    beta_b = np.asarray(beta_b, np.float32)
    Wq = np.asarray(Wq, np.float32)
    Wk = np.asarray(Wk, np.float32)
    Wv = np.asarray(Wv, np.float32)
    Wb = np.asarray(Wb, np.float32)
    Wg = np.asarray(Wg, np.float32)
    bg = np.asarray(bg, np.float32)
    Wout = np.asarray(Wout, np.float32)
    bout = np.asarray(bout, np.float32)

    scaling = 1.0 / math.sqrt(D)
    wq = gamma_p[:, None] * Wq * scaling
    wk = gamma_p[:, None] * Wk / L
    wv = gamma_p[:, None] * Wv
    wg = gamma_p[:, None] * Wg
    # Each core adds the full (AllGathered) bias term into its partial
    # logits before the AllReduce, so the bias must be pre-scaled by 1/8.
    wb = gamma_b[:, None] * Wb / NCORES
    qb = beta_p @ Wq * scaling
    kb = beta_p @ Wk / L
    vb = beta_p @ Wv
    gbf = beta_p @ Wg + bg
    bb = beta_b @ Wb / NCORES
    # packed per-partition bias columns
    bb_pk = np.zeros(128, np.float32)
    cvec = np.zeros((128, 6), np.float32)
    cvec[:, 2] = gbf
    cvec[:, 4] = bout
    cvec[:, 5] = 1.0
    for k4 in range(4):
        for h in range(H):
            bb_pk[32 * k4 + h] = bb[h]
    cvec[:, 3] = bb_pk
    has_qb = bool(np.any(qb != 0))
    has_kb = bool(np.any(kb != 0))
    if has_qb or has_kb:
        # per-head packed columns differ; approximate support by requiring the
        # packed column to be head-independent. With beta_p == 0 (the actual
        # inputs) these are all zero and this path never triggers.
        qh = qb.reshape(H, D)
        kh = kb.reshape(H, D)
        if not (np.allclose(qh, qh[0:1]) and np.allclose(kh, kh[0:1])):
            raise NotImplementedError("head-dependent q/k bias not supported")
        cvec[:, 0] = np.tile(qh[0], 4)
        cvec[:, 1] = np.tile(kh[0], 4)
    vbrow = np.tile(vb, TC)[None, :]
    wbp = np.zeros((DB, 32), np.float32)
    wbp[:, :H] = wb

    nc = build_program(wq, wk, wv, wg, wb, Wout, qb, kb, vb, gbf, bb, bout)

    # ------------------------------------------------------------- shard
    pair_t = np.ascontiguousarray(pair[0].transpose(1, 0, 2))  # [n, t, c]
    bias_t = np.ascontiguousarray(bias[0].transpose(1, 0, 2))  # [i, j, c]
    in_maps = []
    for c in range(NCORES):
        in_maps.append({
            "pair_s": pair_t[c * NLOC:(c + 1) * NLOC].astype(NPBF16),
            "bias_s": bias_t[c * NLOC:(c + 1) * NLOC].astype(NPBF16),
            "wq": wq.astype(NPBF16),
            "wk": wk.astype(NPBF16),
            "wv": wv.astype(NPBF16),
            "wg": wg.astype(NPBF16),
            "wb": wbp.astype(NPBF16),
            "wout": Wout.astype(NPBF16),
            "cvec": cvec,
            "vbrow": vbrow,
            "boutrow": bout[None, :].astype(NPBF16),
            "ident": np.eye(128, dtype=np.float32).astype(NPBF16),
        })
    return nc, in_maps


def assemble(outs):
    """outs: list of 8 per-core [48, 384, 128] arrays -> full [1, L, L, DP]."""
    full = np.concatenate(outs, axis=0)        # [384 n, 384 i, 128]
    final = full.transpose(1, 0, 2)[None]      # [1, i, n, dp] == [1, L, L, DP]
    return np.ascontiguousarray(final, dtype=np.float32)


def kernel(pair, bias, gamma_p, beta_p, gamma_b, beta_b,
           Wq, Wk, Wv, Wb, Wg, bg, Wout, bout):
    nc, in_maps = prepare(pair, bias, gamma_p, beta_p, gamma_b, beta_b,
                          Wq, Wk, Wv, Wb, Wg, bg, Wout, bout)
    if not nc.is_finalized():
        nc.finalize()
    res = run_bass_kernel_spmd(nc, in_maps, list(range(NCORES)))
    global LAST_RESULT
    LAST_RESULT = res
    outs = [res.results[c]["out"] for c in range(NCORES)]  # [48, 384, 128] each
    return assemble(outs)
